# revision 18
# baseline (speedup 1.0000x reference)
"""Binary Conv2d (sign-act 3x3 binary conv + RPReLU + residual) on 8 trn2 NeuronCores.

Reference computation (forward values):
  a  = sign(x + move0_bias)                       # {-1,0,+1}
  bw = scale_o * sign(conv_w), scale_o = mean |conv_w| over (I,KH,KW)
  z  = conv2d(a, bw, pad=1) + pr_bias0
  y  = where(z>=0, z, alpha*z) + pr_bias1 + x

Strategy: data-parallel over batch (16 imgs -> 2 per core). Conv as 9 tap
matmuls with fp8e4 DoubleRow (contracts both 128-channel chunks per matmul,
2 MACs/cell/cycle) accumulating in PSUM; activations are exact sign values
in fp8, stored in a zero-bordered 66-wide padded tile per (img); weights are
sign(w) fp8 (exact).

v2 schedule (vs baseline 91.1us):
- Fused epilogue: scalar-engine Prelu (verified on HW; per-partition
  scale/bias/alpha) computes where(z>=0,z,a*z) from PSUM in ONE op; then a
  single residual add (+x) on vector/gpsimd, which also triggers the y DMA
  on its own queue. Requires pr_bias1 == 0 (true here); general fallback
  keeps the 4-op (1-a)-Relu identity epilogue.
- Startup: DMA triggers split across engine queues (sync: x img0+img1,
  scalar: weights, vector: first x chunk, gpsimd: biases + img1 x) so the
  first activations land ~5us earlier; first x group is 10 rows so sign/MM
  start ASAP; PE warm-up trimmed to 8 dummy matmuls (HAM needs ~3.4us).
- Tail: kernel ends on two single-bank units with the short prelu+add
  chain, adds alternating vector/gpsimd so the final chains overlap.
"""

import sys
for _p in ("/opt/trn_rl_repo",):
    if _p not in sys.path:
        sys.path.append(_p)

import os
from contextlib import ExitStack

import numpy as np
import ml_dtypes

import concourse.bass as bass
import concourse.tile as tile
from concourse import bacc, mybir
from concourse import bass_utils

N_CORES = 8
B, C, H, W = 16, 256, 64, 64
K = 3
BPC = B // N_CORES            # imgs per core
NCH = C // 128                # channel chunks (2)
PW = W + 2                    # padded width 66
PHR = 72                      # padded rows allocated (>=66, CST 16-aligned)
CST = PHR * PW                # per-chunk stride in act tile (4752, %16==0)
SP = H * W                    # spatial 4096
RB = 8                        # out rows per block
NBE = RB * W                  # 512 block elems
PBE = 2 * NBE                 # 1024 elems per pair unit
NTAP = K * K

F32 = mybir.dt.float32
BF16 = mybir.dt.bfloat16
FP8 = mybir.dt.float8e4

N_WARM = int(os.environ.get("K_WARM", "7"))
W_SPLIT = 4 * NCH * NCH * 128   # first w transfer: taps 0-3

# x DMA row groups (first img0 group small for fast start)
GROUPS_B0 = [(0, 9), (9, 17), (17, 33), (33, 49), (49, 64)]
GROUPS_B1 = [(0, 17), (17, 33), (33, 49), (49, 64)]

_CACHE = {}


def _build_program(epi_mode: str, x_dt=None):
    x_dt = BF16 if x_dt is None else x_dt
    nc = bacc.Bacc(
        "TRN2",
        target_bir_lowering=False,
        debug=False,
        enable_asserts=False,
        num_devices=N_CORES,
    )
    # x shipped as bf16: halves HBM traffic + startup latency. sign() of a
    # bf16-rounded value equals sign() of the f32 (rounding keeps sign);
    # the residual add then carries <=2^-9 relative error on the x term,
    # ~5e-4 l2 on y (gate is 2e-2). Requires move0_bias == 0 for the sign
    # path to be exact; caller falls back to f32 x otherwise.
    x_d = nc.dram_tensor("x", [BPC, C, H, W], x_dt, kind="ExternalInput").ap()
    # weight pack: [128, tap(9) * oc(2) * icpair(2) * 128] fp8 sign values
    w_d = nc.dram_tensor("w", [128, NTAP * NCH * NCH * 128], FP8,
                         kind="ExternalInput").ap()
    # per-chunk column packs: mb[p, ic], epi[p, oc*4 + j]
    mb_d = nc.dram_tensor("mb", [128, NCH], F32, kind="ExternalInput").ap()
    # epilogue constants j = 0..3:
    #   prelu mode: [s, b0, alpha, unused]
    #   relu2 mode: [(1-a)s, (1-a)b0, a*s, a*b0 + b1]
    epi_d = nc.dram_tensor("epi", [128, 4 * NCH], F32,
                           kind="ExternalInput").ap()
    y_d = nc.dram_tensor("y", [BPC, C, H, W], F32, kind="ExternalOutput").ap()

    with tile.TileContext(nc) as tc:
        _kernel(tc, y_d, x_d, w_d, mb_d, epi_d, epi_mode, x_dt)
    nc.compile()
    return nc


def _kernel(tc, y_d, x_d, w_d, mb_d, epi_d, epi_mode, x_dt):
    nc = tc.nc
    ctx = ExitStack()
    with ctx:
        const = ctx.enter_context(tc.tile_pool(name="const", bufs=1))
        xpool = ctx.enter_context(tc.tile_pool(name="x", bufs=1))
        apool = ctx.enter_context(tc.tile_pool(name="act", bufs=1))
        work = ctx.enter_context(tc.tile_pool(name="work", bufs=4))
        psum = ctx.enter_context(tc.tile_pool(name="psum", bufs=4, space="PSUM"))

        # --- tiles ---
        x_flat = x_d.rearrange("b c h w -> b c (h w)")
        y_flat = y_d.rearrange("b c h w -> b c (h w)")
        xt = {}   # (b, ic) -> [128, 4096] f32 (residual source)
        at = {}   # b -> [128, 2*CST] fp8 padded sign, chunk ic at offset ic*CST
        for b in range(BPC):
            at[b] = apool.tile([128, NCH * CST], FP8, tag=f"at{b}",
                               name=f"at{b}")
            for ic in range(NCH):
                xt[b, ic] = xpool.tile([128, SP], x_dt, tag=f"xt{b}{ic}",
                                       name=f"xt{b}{ic}")
        wt = const.tile([128, NTAP * NCH * NCH * 128], FP8, tag="wt")
        warm = const.tile([128, NBE], FP8, tag="warm")
        mbt = const.tile([128, NCH], F32, tag="mb", name="mbt")
        ept = const.tile([128, 4 * NCH], F32, tag="ep", name="ept")

        def dma_x_rows(eng, b, ic, r0, r1):
            xs = xt[b, ic][:, r0 * W:r1 * W]
            return eng.dma_start(
                out=xs,
                in_=x_flat[b, ic * 128:(ic + 1) * 128, r0 * W:r1 * W])

        # --- startup DMA triggers, on the two HWDGE rings (sync+scalar;
        # gpsimd DMA is slow ucode SWDGE with an expensive exit drain).
        # sync ring: all x rows (img0 groups in consumption order, then
        # img1); scalar ring: mb pump, then weights split so taps 0-2
        # land before the first matmul needs them. Ring throughput is the
        # startup bottleneck, so nothing else rides in front.
        dma_x_rows(nc.sync, 0, 0, *GROUPS_B0[0])
        nc.sync.dma_start(out=mbt[:], in_=mb_d[:])
        for g in range(1, len(GROUPS_B0)):
            for ic in range(NCH):
                dma_x_rows(nc.sync, 0, ic, *GROUPS_B0[g])
        for g in range(len(GROUPS_B1)):
            for ic in range(NCH):
                dma_x_rows(nc.sync, 1, ic, *GROUPS_B1[g])
        dma_x_rows(nc.scalar, 0, 1, *GROUPS_B0[0])
        nc.scalar.dma_start(out=wt[:, 0:W_SPLIT], in_=w_d[:, 0:W_SPLIT])
        nc.scalar.dma_start(out=wt[:, W_SPLIT:], in_=w_d[:, W_SPLIT:])
        # vector: warm-up operand memset, then img0 borders
        nc.vector.memset(warm[:], 1.0)
        a40 = at[0][:].rearrange("p (i h w) -> p i h w", i=NCH, w=PW)
        nc.vector.memset(a40[:, :, 0:1, :], 0.0)
        nc.vector.memset(a40[:, :, H + 1:H + 2, :], 0.0)
        nc.vector.memset(a40[:, :, 1:H + 1, 0:1], 0.0)
        nc.vector.memset(a40[:, :, 1:H + 1, PW - 1:PW], 0.0)
        # gpsimd: img1 borders only (no DMAs on gpsimd!)
        a41 = at[1][:].rearrange("p (i h w) -> p i h w", i=NCH, w=PW)
        nc.gpsimd.memset(a41[:, :, 0:1, :], 0.0)
        nc.gpsimd.memset(a41[:, :, H + 1:H + 2, :], 0.0)
        nc.gpsimd.memset(a41[:, :, 1:H + 1, 0:1], 0.0)
        nc.gpsimd.memset(a41[:, :, 1:H + 1, PW - 1:PW], 0.0)

        # --- PE warm-up: dummy matmuls release the HAM clock gate (~3.4us
        # of PE activity at cold 1.2GHz) while the startup DMAs land ---
        wps = psum.tile([128, PBE], F32, tag="pt", name="wps")
        for _ in range(N_WARM):
            nc.tensor.matmul(wps[:, 0:NBE], warm[:, 0:128], warm[:],
                             start=True, stop=True)

        # --- img0 sign activations (scalar), in DMA arrival order ---
        def sign_act(b, ic, r0, r1):
            xs = xt[b, ic][:, r0 * W:r1 * W]
            a4 = at[b][:].rearrange("p (i h w) -> p i h w", i=NCH, w=PW)
            x3 = xs.rearrange("p (h w) -> p h w", w=W)
            nc.scalar.activation(
                a4[:, ic, 1 + r0:1 + r1, 1:1 + W], x3,
                mybir.ActivationFunctionType.Sign,
                bias=mbt[:, ic:ic + 1], scale=1.0)

        for g in range(len(GROUPS_B0)):
            for ic in range(NCH):
                sign_act(0, ic, *GROUPS_B0[g])
        # epilogue constants: needed from the first prelu (~15us) on
        nc.scalar.dma_start(out=ept[:], in_=epi_d[:])

        # --- conv units: pairs of 8-row blocks share a 2-bank PSUM tile;
        # the kernel ends on two single blocks so the final (serial)
        # epilogue chain is short. Per-unit epilogue:
        #   prelu mode: yt = Prelu(s*psum + b0); yt += x; dma (2 ops)
        #   relu2 mode: r = Relu((1-a)(s p + b0)); v = a s p + (a b0 + b1);
        #               yt = r + v; yt += x; dma (4 ops)
        # Residual adds alternate vector/gpsimd (tail units on vector so
        # gpsimd's work — and its expensive SWDGE exit drain — finishes
        # early and hides under the matmul stream). y DMA triggers go on
        # the scalar HWDGE ring, delayed one unit behind the prelu so the
        # scalar engine never stalls waiting for the residual add.
        NU = 17
        pending = []  # delayed y DMA triggers: (dst, src)

        def flush_pending():
            while pending:
                dst, src = pending.pop(0)
                nc.scalar.dma_start(out=dst, in_=src)

        ui = 0
        for b in range(BPC):
            a4 = at[b][:].rearrange("p (i h w) -> p i h w", i=NCH, w=PW)
            for oc in range(NCH):
                final_grp = (b == BPC - 1 and oc == NCH - 1)
                if final_grp:
                    units = [(0, 16), (16, 16), (32, 16), (48, 8), (56, 8)]
                else:
                    units = [(0, 16), (16, 16), (32, 16), (48, 16)]
                for (r0u, nru) in units:
                    ube = nru * W
                    pt = psum.tile([128, PBE], F32, tag="pt")
                    off = 0
                    for rb0 in range(r0u, r0u + nru, RB):
                        nr = min(RB, r0u + nru - rb0)
                        out_half = pt[:, off:off + nr * W]
                        off += nr * W
                        for kh in range(K):
                            for kw in range(K):
                                t = kh * K + kw
                                wsl = wt[:, (t * NCH + oc) * NCH
                                         * 128:(t * NCH + oc + 1) * NCH * 128]
                                lhsT = wsl.rearrange("p (i m) -> p i m", i=NCH)
                                rhs = a4[:, :, rb0 + kh:rb0 + kh + nr,
                                         kw:kw + W]
                                nc.tensor.matmul(
                                    out_half, lhsT, rhs,
                                    start=(t == 0), stop=(t == NTAP - 1),
                                    perf_mode=mybir.MatmulPerfMode.DoubleRow)
                    eb = 4 * oc
                    base = r0u * W
                    sl = slice(0, ube)
                    xsl = xt[b, oc][:, base:base + ube]
                    yt = work.tile([128, PBE], F32, tag="yt")
                    if epi_mode == "prelu":
                        nc.scalar.activation(
                            yt[:, sl], pt[:, sl],
                            mybir.ActivationFunctionType.Prelu,
                            bias=ept[:, eb + 1:eb + 2],
                            scale=ept[:, eb:eb + 1],
                            alpha=ept[:, eb + 2:eb + 3])
                    else:
                        nc.scalar.activation(
                            yt[:, sl], pt[:, sl],
                            mybir.ActivationFunctionType.Relu,
                            bias=ept[:, eb + 1:eb + 2],
                            scale=ept[:, eb:eb + 1])
                        v = work.tile([128, ube], F32, tag="v", name="v")
                        nc.vector.tensor_scalar(
                            out=v[:], in0=pt[:, sl],
                            scalar1=ept[:, eb + 2:eb + 3],
                            scalar2=ept[:, eb + 3:eb + 4],
                            op0=mybir.AluOpType.mult,
                            op1=mybir.AluOpType.add)
                        nc.vector.tensor_add(out=yt[:, sl], in0=yt[:, sl],
                                             in1=v[:])
                    flush_pending()
                    use_vec = (ui % 2 == 0) or (ui >= NU - 3)
                    add_eng = nc.vector if use_vec else nc.gpsimd
                    add_eng.tensor_add(out=yt[:, sl], in0=yt[:, sl], in1=xsl)
                    pending.append((
                        y_flat[b, oc * 128:(oc + 1) * 128, base:base + ube],
                        yt[:, sl]))
                    ui += 1
                    # weave img1 sign activations into the scalar stream
                    # early enough that img1 acts are ready ~41us (first
                    # img1 matmul) but late enough that their x rows have
                    # landed (~13-25us on the sync ring)
                    if 2 <= ui <= len(GROUPS_B1) + 1:
                        g = ui - 2
                        for ic in range(NCH):
                            sign_act(1, ic, *GROUPS_B1[g])
        flush_pending()


def _pack_inputs(x, move0_bias, conv_w, pr_bias0, prelu_alpha, pr_bias1):
    """Host-side prep: weight binarization + epilogue constant folding."""
    f32 = np.float32
    w = conv_w.astype(f32)
    scale = np.abs(w).mean(axis=(1, 2, 3)).astype(f32)          # (O,)
    ws = np.sign(w).astype(ml_dtypes.float8_e4m3)               # (O,I,KH,KW)
    # lhsT[k=p, tap, oc, ic, m] = ws[oc*128+m, ic*128+p, kh, kw]
    wsr = ws.reshape(NCH, 128, NCH, 128, NTAP)                  # (oc,m,ic,p,t)
    lhsT = wsr.transpose(3, 4, 0, 2, 1)                         # (p,t,oc,ic,m)
    lhsT = np.ascontiguousarray(lhsT).reshape(128, NTAP * NCH * NCH * 128)

    alpha = prelu_alpha.astype(f32).reshape(C)
    b0 = pr_bias0.astype(f32).reshape(C)
    b1 = pr_bias1.astype(f32).reshape(C)
    if np.all(b1 == 0.0):
        epi_mode = "prelu"
        epi = np.stack([scale, b0, alpha, b1], axis=1).astype(f32)
    else:
        assert np.all(alpha < 1.0)
        epi_mode = "relu2"
        epi = np.stack([(1 - alpha) * scale, (1 - alpha) * b0,
                        alpha * scale, alpha * b0 + b1], axis=1).astype(f32)
    # column packs: epi[C,4] -> [128, oc*4+j]; mb[C] -> [128, ic]
    epi_p = np.ascontiguousarray(
        epi.reshape(NCH, 128, 4).transpose(1, 0, 2).reshape(128, 4 * NCH))
    mb = move0_bias.astype(f32).reshape(C)
    mb_p = np.ascontiguousarray(mb.reshape(NCH, 128).T)
    # bf16 x is safe when move0_bias == 0: bf16 rounding preserves sign(x),
    # and the residual-add error (~5e-4 l2 on y) is far under the 2e-2 gate
    if np.all(mb == 0.0):
        x_np, x_dt = ml_dtypes.bfloat16, "bf16"
    else:
        x_np, x_dt = f32, "f32"

    common = {"w": lhsT, "mb": mb_p, "epi": epi_p}
    in_maps = []
    for i in range(N_CORES):
        m = dict(common)
        m["x"] = np.ascontiguousarray(x[i * BPC:(i + 1) * BPC].astype(x_np))
        in_maps.append(m)
    return in_maps, epi_mode, x_dt


def kernel(x, move0_bias, conv_w, pr_bias0, prelu_alpha, pr_bias1):
    in_maps, epi_mode, x_dt = _pack_inputs(
        np.asarray(x), np.asarray(move0_bias), np.asarray(conv_w),
        np.asarray(pr_bias0), np.asarray(prelu_alpha), np.asarray(pr_bias1))
    key = ("nc", epi_mode, x_dt)
    if key not in _CACHE:
        _CACHE[key] = _build_program(
            epi_mode, BF16 if x_dt == "bf16" else F32)
    nc = _CACHE[key]
    res = bass_utils.run_bass_kernel_spmd(nc, in_maps,
                                          core_ids=list(range(N_CORES)))
    _CACHE["last_results"] = res
    out = np.concatenate([res.results[i]["y"] for i in range(N_CORES)], axis=0)
    return out


# revision 23
# speedup vs baseline: 1.0352x; 1.0352x over previous
"""Binary Conv2d (sign-act 3x3 binary conv + RPReLU + residual) on 8 trn2 NeuronCores.

Reference computation (forward values):
  a  = sign(x + move0_bias)                       # {-1,0,+1}
  bw = scale_o * sign(conv_w), scale_o = mean |conv_w| over (I,KH,KW)
  z  = conv2d(a, bw, pad=1) + pr_bias0
  y  = where(z>=0, z, alpha*z) + pr_bias1 + x

Strategy: data-parallel over batch (16 imgs -> 2 per core). Conv as 9 tap
matmuls with fp8e4 DoubleRow (contracts both 128-channel chunks per matmul,
2 MACs/cell/cycle) accumulating in PSUM; activations are exact sign values
in fp8, stored in a zero-bordered 66-wide padded tile per (img); weights are
sign(w) fp8 (exact).

v2 schedule (vs baseline 91.1us):
- Fused epilogue: scalar-engine Prelu (verified on HW; per-partition
  scale/bias/alpha) computes where(z>=0,z,a*z) from PSUM in ONE op; then a
  single residual add (+x) on vector/gpsimd, which also triggers the y DMA
  on its own queue. Requires pr_bias1 == 0 (true here); general fallback
  keeps the 4-op (1-a)-Relu identity epilogue.
- Startup: DMA triggers split across engine queues (sync: x img0+img1,
  scalar: weights, vector: first x chunk, gpsimd: biases + img1 x) so the
  first activations land ~5us earlier; first x group is 10 rows so sign/MM
  start ASAP; PE warm-up trimmed to 8 dummy matmuls (HAM needs ~3.4us).
- Tail: kernel ends on two single-bank units with the short prelu+add
  chain, adds alternating vector/gpsimd so the final chains overlap.
"""

import sys
for _p in ("/opt/trn_rl_repo",):
    if _p not in sys.path:
        sys.path.append(_p)

import os
from contextlib import ExitStack

import numpy as np
import ml_dtypes

import concourse.bass as bass
import concourse.tile as tile
from concourse import bacc, mybir
from concourse import bass_utils

N_CORES = 8
B, C, H, W = 16, 256, 64, 64
K = 3
BPC = B // N_CORES            # imgs per core
NCH = C // 128                # channel chunks (2)
PW = W + 2                    # padded width 66
PHR = 72                      # padded rows allocated (>=66, CST 16-aligned)
CST = PHR * PW                # per-chunk stride in act tile (4752, %16==0)
SP = H * W                    # spatial 4096
RB = 8                        # out rows per block
NBE = RB * W                  # 512 block elems
PBE = 2 * NBE                 # 1024 elems per pair unit
NTAP = K * K

F32 = mybir.dt.float32
BF16 = mybir.dt.bfloat16
FP8 = mybir.dt.float8e4

N_WARM = int(os.environ.get("K_WARM", "7"))
W_SPLIT = 4 * NCH * NCH * 128   # first w transfer: taps 0-3

# x DMA row groups (img0 fine-grained so sign/matmul start ASAP and the
# act pipeline never starves the PE; img1 is needed ~30us later)
GROUPS_B0 = [(0, 9), (9, 17), (17, 25), (25, 33), (33, 41), (41, 49),
             (49, 57), (57, 64)]
GROUPS_B1 = [(0, 17), (17, 33), (33, 49), (49, 64)]

_CACHE = {}


def _build_program(epi_mode: str, x_dt=None):
    x_dt = BF16 if x_dt is None else x_dt
    nc = bacc.Bacc(
        "TRN2",
        target_bir_lowering=False,
        debug=False,
        enable_asserts=False,
        num_devices=N_CORES,
    )
    # x shipped as bf16: halves HBM traffic + startup latency. sign() of a
    # bf16-rounded value equals sign() of the f32 (rounding keeps sign);
    # the residual add then carries <=2^-9 relative error on the x term,
    # ~5e-4 l2 on y (gate is 2e-2). Requires move0_bias == 0 for the sign
    # path to be exact; caller falls back to f32 x otherwise.
    x_d = nc.dram_tensor("x", [BPC, C, H, W], x_dt, kind="ExternalInput").ap()
    # weight pack: [128, tap(9) * oc(2) * icpair(2) * 128] fp8 sign values
    w_d = nc.dram_tensor("w", [128, NTAP * NCH * NCH * 128], FP8,
                         kind="ExternalInput").ap()
    # per-chunk column packs: mb[p, ic], epi[p, oc*4 + j]
    mb_d = nc.dram_tensor("mb", [128, NCH], F32, kind="ExternalInput").ap()
    # epilogue constants j = 0..3:
    #   prelu mode: [s, b0, alpha, unused]
    #   relu2 mode: [(1-a)s, (1-a)b0, a*s, a*b0 + b1]
    epi_d = nc.dram_tensor("epi", [128, 4 * NCH], F32,
                           kind="ExternalInput").ap()
    y_d = nc.dram_tensor("y", [BPC, C, H, W], F32, kind="ExternalOutput").ap()

    with tile.TileContext(nc) as tc:
        _kernel(tc, y_d, x_d, w_d, mb_d, epi_d, epi_mode, x_dt)
    nc.compile()
    return nc


def _kernel(tc, y_d, x_d, w_d, mb_d, epi_d, epi_mode, x_dt):
    nc = tc.nc
    ctx = ExitStack()
    with ctx:
        const = ctx.enter_context(tc.tile_pool(name="const", bufs=1))
        xpool = ctx.enter_context(tc.tile_pool(name="x", bufs=1))
        apool = ctx.enter_context(tc.tile_pool(name="act", bufs=1))
        work = ctx.enter_context(tc.tile_pool(name="work", bufs=6))
        psum = ctx.enter_context(tc.tile_pool(name="psum", bufs=4, space="PSUM"))

        # --- tiles ---
        x_flat = x_d.rearrange("b c h w -> b c (h w)")
        y_flat = y_d.rearrange("b c h w -> b c (h w)")
        xt = {}   # (b, ic) -> [128, 4096] f32 (residual source)
        at = {}   # b -> [128, 2*CST] fp8 padded sign, chunk ic at offset ic*CST
        for b in range(BPC):
            at[b] = apool.tile([128, NCH * CST], FP8, tag=f"at{b}",
                               name=f"at{b}")
            for ic in range(NCH):
                xt[b, ic] = xpool.tile([128, SP], x_dt, tag=f"xt{b}{ic}",
                                       name=f"xt{b}{ic}")
        wt = const.tile([128, NTAP * NCH * NCH * 128], FP8, tag="wt")
        warm = const.tile([128, NBE], FP8, tag="warm")
        mbt = const.tile([128, NCH], F32, tag="mb", name="mbt")
        ept = const.tile([128, 4 * NCH], F32, tag="ep", name="ept")

        def dma_x_rows(eng, b, ic, r0, r1):
            xs = xt[b, ic][:, r0 * W:r1 * W]
            return eng.dma_start(
                out=xs,
                in_=x_flat[b, ic * 128:(ic + 1) * 128, r0 * W:r1 * W])

        # --- startup DMA triggers, on the two HWDGE rings (sync+scalar;
        # gpsimd DMA is slow ucode SWDGE with an expensive exit drain).
        # Notes from traces: a DGE ring holds only ~4-5 outstanding
        # transfers, a stalled trigger blocks its whole engine queue, and
        # the Tile scheduler happily hoists no-dep triggers over
        # not-yet-ready compute — so the scalar engine carries ONLY the
        # two weight transfers (its auto ACT_TABLE_LOAD runs first
        # anyway), and everything else rides the sync ring in exact
        # need-order. Weights split so taps 0-3 land before the first
        # matmul and taps 4-8 one tap-stream later.
        nc.scalar.dma_start(out=wt[:, 0:W_SPLIT], in_=w_d[:, 0:W_SPLIT])
        nc.scalar.dma_start(out=wt[:, W_SPLIT:], in_=w_d[:, W_SPLIT:])
        dma_x_rows(nc.sync, 0, 0, *GROUPS_B0[0])
        dma_x_rows(nc.sync, 0, 1, *GROUPS_B0[0])
        nc.sync.dma_start(out=mbt[:], in_=mb_d[:])
        for g in range(1, len(GROUPS_B0)):
            for ic in range(NCH):
                dma_x_rows(nc.sync, 0, ic, *GROUPS_B0[g])
            if g == 2:
                nc.sync.dma_start(out=ept[:], in_=epi_d[:])
        for g in range(len(GROUPS_B1)):
            for ic in range(NCH):
                dma_x_rows(nc.sync, 1, ic, *GROUPS_B1[g])
        # vector: warm-up operand memset, then img0 borders
        nc.vector.memset(warm[:], 1.0)
        a40 = at[0][:].rearrange("p (i h w) -> p i h w", i=NCH, w=PW)
        nc.vector.memset(a40[:, :, 0:1, :], 0.0)
        nc.vector.memset(a40[:, :, H + 1:H + 2, :], 0.0)
        nc.vector.memset(a40[:, :, 1:H + 1, 0:1], 0.0)
        nc.vector.memset(a40[:, :, 1:H + 1, PW - 1:PW], 0.0)
        # gpsimd: img1 borders only (no DMAs on gpsimd!)
        a41 = at[1][:].rearrange("p (i h w) -> p i h w", i=NCH, w=PW)
        nc.gpsimd.memset(a41[:, :, 0:1, :], 0.0)
        nc.gpsimd.memset(a41[:, :, H + 1:H + 2, :], 0.0)
        nc.gpsimd.memset(a41[:, :, 1:H + 1, 0:1], 0.0)
        nc.gpsimd.memset(a41[:, :, 1:H + 1, PW - 1:PW], 0.0)

        # --- PE warm-up: dummy matmuls release the HAM clock gate (~3.4us
        # of PE activity at cold 1.2GHz) while the startup DMAs land ---
        wps = psum.tile([128, PBE], F32, tag="pt", name="wps")
        for _ in range(N_WARM):
            nc.tensor.matmul(wps[:, 0:NBE], warm[:, 0:128], warm[:],
                             start=True, stop=True)

        # --- img0 sign activations (scalar), in DMA arrival order ---
        def sign_act(b, ic, r0, r1):
            xs = xt[b, ic][:, r0 * W:r1 * W]
            a4 = at[b][:].rearrange("p (i h w) -> p i h w", i=NCH, w=PW)
            x3 = xs.rearrange("p (h w) -> p h w", w=W)
            nc.scalar.activation(
                a4[:, ic, 1 + r0:1 + r1, 1:1 + W], x3,
                mybir.ActivationFunctionType.Sign,
                bias=mbt[:, ic:ic + 1], scale=1.0)

        for g in range(len(GROUPS_B0)):
            for ic in range(NCH):
                sign_act(0, ic, *GROUPS_B0[g])

        # --- conv units: pairs of 8-row blocks share a 2-bank PSUM tile;
        # the kernel ends on two single blocks so the final (serial)
        # epilogue chain is short. Per-unit epilogue:
        #   prelu mode: yt = Prelu(s*psum + b0); yt += x; dma (2 ops)
        #   relu2 mode: r = Relu((1-a)(s p + b0)); v = a s p + (a b0 + b1);
        #               yt = r + v; yt += x; dma (4 ops)
        # Residual adds alternate vector/gpsimd (tail units on vector so
        # gpsimd's work — and its expensive SWDGE exit drain — finishes
        # early and hides under the matmul stream). y DMA triggers ride
        # the sync ring behind all input transfers, in unit order.
        NU = 17
        pending = []  # delayed y DMA triggers: (dst, src)

        def flush_pending():
            while pending:
                dst, src = pending.pop(0)
                nc.sync.dma_start(out=dst, in_=src)

        ui = 0
        for b in range(BPC):
            a4 = at[b][:].rearrange("p (i h w) -> p i h w", i=NCH, w=PW)
            for oc in range(NCH):
                final_grp = (b == BPC - 1 and oc == NCH - 1)
                if final_grp:
                    units = [(0, 16), (16, 16), (32, 16), (48, 8), (56, 8)]
                else:
                    units = [(0, 16), (16, 16), (32, 16), (48, 16)]
                for (r0u, nru) in units:
                    ube = nru * W
                    pt = psum.tile([128, PBE], F32, tag="pt")
                    off = 0
                    for rb0 in range(r0u, r0u + nru, RB):
                        nr = min(RB, r0u + nru - rb0)
                        out_half = pt[:, off:off + nr * W]
                        off += nr * W
                        for kh in range(K):
                            for kw in range(K):
                                t = kh * K + kw
                                wsl = wt[:, (t * NCH + oc) * NCH
                                         * 128:(t * NCH + oc + 1) * NCH * 128]
                                lhsT = wsl.rearrange("p (i m) -> p i m", i=NCH)
                                rhs = a4[:, :, rb0 + kh:rb0 + kh + nr,
                                         kw:kw + W]
                                nc.tensor.matmul(
                                    out_half, lhsT, rhs,
                                    start=(t == 0), stop=(t == NTAP - 1),
                                    perf_mode=mybir.MatmulPerfMode.DoubleRow)
                    eb = 4 * oc
                    base = r0u * W
                    sl = slice(0, ube)
                    xsl = xt[b, oc][:, base:base + ube]
                    yt = work.tile([128, PBE], F32, tag="yt")
                    if epi_mode == "prelu":
                        nc.scalar.activation(
                            yt[:, sl], pt[:, sl],
                            mybir.ActivationFunctionType.Prelu,
                            bias=ept[:, eb + 1:eb + 2],
                            scale=ept[:, eb:eb + 1],
                            alpha=ept[:, eb + 2:eb + 3])
                    else:
                        nc.scalar.activation(
                            yt[:, sl], pt[:, sl],
                            mybir.ActivationFunctionType.Relu,
                            bias=ept[:, eb + 1:eb + 2],
                            scale=ept[:, eb:eb + 1])
                        v = work.tile([128, ube], F32, tag="v", name="v")
                        nc.vector.tensor_scalar(
                            out=v[:], in0=pt[:, sl],
                            scalar1=ept[:, eb + 2:eb + 3],
                            scalar2=ept[:, eb + 3:eb + 4],
                            op0=mybir.AluOpType.mult,
                            op1=mybir.AluOpType.add)
                        nc.vector.tensor_add(out=yt[:, sl], in0=yt[:, sl],
                                             in1=v[:])
                    flush_pending()
                    use_vec = (ui % 2 == 0) or (ui >= NU - 3)
                    add_eng = nc.vector if use_vec else nc.gpsimd
                    add_eng.tensor_add(out=yt[:, sl], in0=yt[:, sl], in1=xsl)
                    pending.append((
                        y_flat[b, oc * 128:(oc + 1) * 128, base:base + ube],
                        yt[:, sl]))
                    ui += 1
                    # weave img1 sign activations into the scalar stream
                    # early enough that img1 acts are ready ~41us (first
                    # img1 matmul) but late enough that their x rows have
                    # landed (~13-25us on the sync ring)
                    if 2 <= ui <= len(GROUPS_B1) + 1:
                        g = ui - 2
                        for ic in range(NCH):
                            sign_act(1, ic, *GROUPS_B1[g])
        flush_pending()


def _pack_inputs(x, move0_bias, conv_w, pr_bias0, prelu_alpha, pr_bias1):
    """Host-side prep: weight binarization + epilogue constant folding."""
    f32 = np.float32
    w = conv_w.astype(f32)
    scale = np.abs(w).mean(axis=(1, 2, 3)).astype(f32)          # (O,)
    ws = np.sign(w).astype(ml_dtypes.float8_e4m3)               # (O,I,KH,KW)
    # lhsT[k=p, tap, oc, ic, m] = ws[oc*128+m, ic*128+p, kh, kw]
    wsr = ws.reshape(NCH, 128, NCH, 128, NTAP)                  # (oc,m,ic,p,t)
    lhsT = wsr.transpose(3, 4, 0, 2, 1)                         # (p,t,oc,ic,m)
    lhsT = np.ascontiguousarray(lhsT).reshape(128, NTAP * NCH * NCH * 128)

    alpha = prelu_alpha.astype(f32).reshape(C)
    b0 = pr_bias0.astype(f32).reshape(C)
    b1 = pr_bias1.astype(f32).reshape(C)
    if np.all(b1 == 0.0):
        epi_mode = "prelu"
        epi = np.stack([scale, b0, alpha, b1], axis=1).astype(f32)
    else:
        assert np.all(alpha < 1.0)
        epi_mode = "relu2"
        epi = np.stack([(1 - alpha) * scale, (1 - alpha) * b0,
                        alpha * scale, alpha * b0 + b1], axis=1).astype(f32)
    # column packs: epi[C,4] -> [128, oc*4+j]; mb[C] -> [128, ic]
    epi_p = np.ascontiguousarray(
        epi.reshape(NCH, 128, 4).transpose(1, 0, 2).reshape(128, 4 * NCH))
    mb = move0_bias.astype(f32).reshape(C)
    mb_p = np.ascontiguousarray(mb.reshape(NCH, 128).T)
    # bf16 x is safe when move0_bias == 0: bf16 rounding preserves sign(x),
    # and the residual-add error (~5e-4 l2 on y) is far under the 2e-2 gate
    if np.all(mb == 0.0):
        x_np, x_dt = ml_dtypes.bfloat16, "bf16"
    else:
        x_np, x_dt = f32, "f32"

    common = {"w": lhsT, "mb": mb_p, "epi": epi_p}
    in_maps = []
    for i in range(N_CORES):
        m = dict(common)
        m["x"] = np.ascontiguousarray(x[i * BPC:(i + 1) * BPC].astype(x_np))
        in_maps.append(m)
    return in_maps, epi_mode, x_dt


def kernel(x, move0_bias, conv_w, pr_bias0, prelu_alpha, pr_bias1):
    in_maps, epi_mode, x_dt = _pack_inputs(
        np.asarray(x), np.asarray(move0_bias), np.asarray(conv_w),
        np.asarray(pr_bias0), np.asarray(prelu_alpha), np.asarray(pr_bias1))
    key = ("nc", epi_mode, x_dt)
    if key not in _CACHE:
        _CACHE[key] = _build_program(
            epi_mode, BF16 if x_dt == "bf16" else F32)
    nc = _CACHE[key]
    res = bass_utils.run_bass_kernel_spmd(nc, in_maps,
                                          core_ids=list(range(N_CORES)))
    _CACHE["last_results"] = res
    out = np.concatenate([res.results[i]["y"] for i in range(N_CORES)], axis=0)
    return out
